# revision 2
# baseline (speedup 1.0000x reference)
"""Linear attention (elu(x)+1 feature map) Bass/Tile kernel for Trainium2.

Problem: B=4, H=16, S=4096, D=64, fp32.
  Qf = elu(Q)+1; Kf = elu(K)+1
  KV = Kf^T (V*mask);  Ksum = Kf^T mask
  out = (Qf @ KV) / (Qf . Ksum)

Sharding: 64 (b,h) pairs data-parallel over 8 cores, 8 pairs/core, no
collectives. Pairs processed in 4 groups of 2 so matmuls use full 128
partitions.

v6 design (timeline: v1 384us -> v2 346 -> v3 117 -> v4 95 -> v5 86):
The v5 trace showed DMA input streaming finishes at ~45us (full 360GB/s)
but compute drags to 88us: DVE 67% / ACT 61% busy - the elu feature map
(exp+relu on ACT, min+add on DVE) and the fp32-PSUM normalization were
the bottleneck, plus 640 matmul+LDWEIGHTS pairs. v6 changes:
- Q is loaded pre-transposed via XBAR DMA-transpose (HWDGE): the host
  permutes Q rows to (blk, j, p) order so the transposed SBUF tile IS
  the lhsT layout phase B needs. Kills 32 PE transposes/group, the
  bf16 PSUM transpose tiles, and lets Q's feature map run on SBUF
  operands (4x DVE mode) instead of PSUM.
- Feature map via the identity elu(x)+1 = min(exp(x), 1+relu(x)), with
  1+relu(x) = max(x+1, 1) as ONE dual-op tensor_scalar on DVE. ACT now
  does only exp (2 ops each for Q/K per group); DVE does TS+TT at 4x
  bf16 SBUF rate. ACT work halves, DVE feature work drops ~35%.
- KV accumulation: one 130-col matmul per (blk, j) over the pair-
  interleaved [K0|K1] x [V0|m0|V1|m1] tiles (cross-pair blocks land in
  unused PSUM columns), halving KV matmul+LDWEIGHTS count.
- K/V keep the s = blk*1024 + p*8 + j interleave (2KiB DMA runs); Q/out
  use s = blk*1024 + j*128 + p (phase-B partition mapping) - legal
  because KV/Ksum are s-reductions, so the two orders never meet
  elementwise. Output is stored pair-interleaved [s, u, d] (2KiB runs)
  and deinterleaved on the host.
- Normalization: one TT mult per block with a 3-dim strided+broadcast
  rec AP; reciprocal once per block from the den PSUM pair-columns.
- Queues: sync HWDGE = K loads; scalar HWDGE = Q transpose-loads + out
  stores; gpsimd SWDGE = V loads. Loads never queue behind stores.
"""

import numpy as np

import concourse.bass as bass
import concourse.mybir as mybir
import concourse.tile as tile
from concourse.bass_utils import run_bass_kernel_spmd

F32 = mybir.dt.float32
BF16 = mybir.dt.bfloat16
AF = mybir.ActivationFunctionType
ALU = mybir.AluOpType

N_CORES = 8
PAIRS = 8          # (b,h) pairs per core
S = 4096
D = 64
E = D + 1          # V is host-padded with the mask column
E2 = 2 * E         # both pairs' V columns in one rhs
NB = 4             # blocks (of 1024 rows) per pair
NJ = 8             # K/V: s = blk*1024 + p*8 + j ; Q/O: s = blk*1024 + j*128 + p
NG = PAIRS // 2    # pair-groups
NH = 2             # half-groups (2 blocks each) per group


def build_bass() -> bass.Bass:
    from contextlib import ExitStack
    from concourse.bacc import Bacc
    nc = Bacc()
    # host layouts (see module docstring for the s-orderings)
    Qh = nc.dram_tensor("Q", [NG, S, 128], BF16, kind="ExternalInput")
    Kh = nc.dram_tensor("K", [NG, S, 2, D], BF16, kind="ExternalInput")
    Vh = nc.dram_tensor("V", [NG, S, 2, E], BF16, kind="ExternalInput")
    Oh = nc.dram_tensor("O", [NG, S, 2, D], BF16, kind="ExternalOutput")

    # K/V: s = h*2048 + c*1024 + p*8 + j
    Kv = [Kh[g].rearrange("(h c p j) u d -> h p c j u d",
                          h=NH, c=2, p=128, j=NJ) for g in range(NG)]
    Vv = [Vh[g].rearrange("(b p j) u e -> p b j u e", b=NB, p=128, j=NJ)
          for g in range(NG)]
    # O: out partition p <-> original s = blk*1024 + p*8 + j (host maps back)
    Ov = [Oh[g].rearrange("(b p j) u d -> p b j u d", b=NB, p=128, j=NJ)
          for g in range(NG)]

    with tile.TileContext(nc) as tc, ExitStack() as ctx, \
            nc.allow_low_precision("bf16 pipeline; fro gate is 2e-2"):
        kr_pool = ctx.enter_context(tc.tile_pool(name="kr", bufs=4))
        exk_pool = ctx.enter_context(tc.tile_pool(name="exk", bufs=2))
        rk_pool = ctx.enter_context(tc.tile_pool(name="rk", bufs=2))
        kf_pool = ctx.enter_context(tc.tile_pool(name="kf", bufs=2))
        vm_pool = ctx.enter_context(tc.tile_pool(name="vm", bufs=2))
        qt_pool = ctx.enter_context(tc.tile_pool(name="qt", bufs=2))
        exq_pool = ctx.enter_context(tc.tile_pool(name="exq", bufs=2))
        rq_pool = ctx.enter_context(tc.tile_pool(name="rq", bufs=2))
        qtf_pool = ctx.enter_context(tc.tile_pool(name="qtf", bufs=2))
        bd_pool = ctx.enter_context(tc.tile_pool(name="bd", bufs=2))
        ks_pool = ctx.enter_context(tc.tile_pool(name="ks", bufs=2))
        rec_pool = ctx.enter_context(tc.tile_pool(name="rec", bufs=2))
        osb_pool = ctx.enter_context(tc.tile_pool(name="osb", bufs=2))
        kv_psum = ctx.enter_context(tc.tile_pool(name="kvps", bufs=2, space="PSUM"))
        ob_psum = ctx.enter_context(tc.tile_pool(name="obps", bufs=2, space="PSUM"))
        dn_psum = ctx.enter_context(tc.tile_pool(name="dnps", bufs=2, space="PSUM"))

        kv_ps_g = [None] * NG
        qtf_g = [None] * NG
        bd_g = [None] * NG
        ks2_g = [None] * NG

        def phase_a(g):
            # generator: 4 yields (one per block)
            kv_ps = kv_psum.tile([128, E2], F32, tag="kv", name=f"kv_{g}")
            kv_ps_g[g] = kv_ps
            vm = vm_pool.tile([128, NB, NJ, 2, E], BF16, tag="vm",
                              name=f"vm_{g}")
            qt = qt_pool.tile([128, NB, NJ, 128], BF16, tag="qt",
                              name=f"qt_{g}")
            qtf = qtf_pool.tile([128, NB, NJ, 128], BF16, tag="qtf",
                                name=f"qtf_{g}")
            qtf_g[g] = qtf
            # whole-group Q arrives transposed via the XBAR (scalar HWDGE)
            nc.scalar.dma_start(out=qt, in_=Qh[g], transpose=True)

            for h in range(NH):
                b0 = 2 * h
                kraw = kr_pool.tile([128, 2, NJ, 2, D], BF16, tag="kr",
                                    name=f"kr_{g}_{h}")
                nc.sync.dma_start(out=kraw, in_=Kv[g][h])
                nc.gpsimd.dma_start(out=vm[:, b0:b0 + 2],
                                    in_=Vv[g][:, b0:b0 + 2])

                # kf = min(exp(K), max(K+1, 1))  [= elu(K)+1]
                exk = exk_pool.tile([128, 2, NJ, 2, D], BF16, tag="exk",
                                    name=f"exk_{g}_{h}")
                rk = rk_pool.tile([128, 2, NJ, 2, D], BF16, tag="rk",
                                  name=f"rk_{g}_{h}")
                kf = kf_pool.tile([128, 2, NJ, 2, D], BF16, tag="kf",
                                  name=f"kf_{g}_{h}")
                nc.scalar.activation(exk, kraw, AF.Exp)
                nc.vector.tensor_scalar(out=rk, in0=kraw, scalar1=1.0,
                                        scalar2=1.0, op0=ALU.add, op1=ALU.max)
                nc.vector.tensor_tensor(out=kf, in0=exk, in1=rk, op=ALU.min)

                for c in range(2):
                    blk = b0 + c
                    # KV accumulation: both pairs in one 130-col matmul
                    for j in range(NJ):
                        cc = blk * NJ + j
                        nc.tensor.matmul(
                            kv_ps, lhsT=kf[:, c, j], rhs=vm[:, blk, j],
                            start=(cc == 0), stop=(cc == NB * NJ - 1),
                            skip_group_check=True)
                    if c == 1:
                        # Q feature map for this half (Q dma long done)
                        exq = exq_pool.tile([128, 2, NJ, 128], BF16,
                                            tag="exq", name=f"exq_{g}_{h}")
                        rq = rq_pool.tile([128, 2, NJ, 128], BF16,
                                          tag="rq", name=f"rq_{g}_{h}")
                        nc.scalar.activation(exq, qt[:, b0:b0 + 2], AF.Exp)
                        nc.vector.tensor_scalar(out=rq, in0=qt[:, b0:b0 + 2],
                                                scalar1=1.0, scalar2=1.0,
                                                op0=ALU.add, op1=ALU.max)
                        nc.vector.tensor_tensor(out=qtf[:, b0:b0 + 2],
                                                in0=exq, in1=rq, op=ALU.min)
                    yield

        def extract_bd(g):
            kv_ps = kv_ps_g[g]
            bd = bd_pool.tile([128, 128], BF16, tag="bd", name=f"bd_{g}")
            ks2 = ks_pool.tile([128, 2], BF16, tag="ks2", name=f"ks2_{g}")
            nc.gpsimd.memset(bd, 0.0)
            nc.gpsimd.memset(ks2, 0.0)
            nc.vector.tensor_copy(out=bd[0:64, 0:64], in_=kv_ps[0:64, 0:D])
            nc.vector.tensor_copy(out=bd[64:128, 64:128],
                                  in_=kv_ps[64:128, E:E + D])
            nc.vector.tensor_copy(out=ks2[0:64, 0:1], in_=kv_ps[0:64, D:E])
            nc.vector.tensor_copy(out=ks2[64:128, 1:2],
                                  in_=kv_ps[64:128, E + D:E2])
            bd_g[g], ks2_g[g] = bd, ks2

        def phase_b(g):
            bd, ks2 = bd_g[g], ks2_g[g]
            qtf = qtf_g[g]
            osb = osb_pool.tile([128, NB, NJ, 2, D], BF16, tag="osb",
                                name=f"osb_{g}")
            for blk in range(NB):
                if blk == 2:
                    nc.scalar.dma_start(out=Ov[g][:, 0:2], in_=osb[:, 0:2])
                if blk > 0:
                    yield
                ob = ob_psum.tile([128, NJ, 128], F32, tag="ob",
                                  name=f"ob_{g}_{blk}")
                dn = dn_psum.tile([128, NJ, 2], F32, tag="dn",
                                  name=f"dn_{g}_{blk}")
                for j in range(NJ):
                    lhsT = qtf[:, blk, j]
                    nc.tensor.matmul(ob[:, j], lhsT=lhsT, rhs=bd,
                                     start=True, stop=True,
                                     skip_group_check=True)
                    nc.tensor.matmul(dn[:, j], lhsT=lhsT, rhs=ks2,
                                     start=True, stop=True,
                                     skip_group_check=True)
                rec = rec_pool.tile([128, 2, NJ], BF16, tag="rec",
                                    name=f"rec_{g}_{blk}")
                nc.vector.reciprocal(rec.rearrange("p u j -> p j u"), dn)
                nc.vector.tensor_tensor(
                    out=osb[:, blk],
                    in0=ob.rearrange("p j (u d) -> p j u d", u=2),
                    in1=rec.rearrange("p u j -> p j u")
                        .to_broadcast([128, NJ, 2, D]),
                    op=ALU.mult)
            nc.scalar.dma_start(out=Ov[g][:, 2:4], in_=osb[:, 2:4])
            yield

        # emission: weave B(g-1) block-chunks 1:1 between A(g) blocks
        a_gens = [phase_a(g) for g in range(NG)]
        b_gens = [None] * NG

        def run(gen):
            if gen is not None:
                next(gen, None)

        for _ in range(NB):
            run(a_gens[0])
        extract_bd(0)
        b_gens[0] = phase_b(0)
        for g in range(1, NG):
            for blk in range(NB):
                run(a_gens[g])
                run(b_gens[g - 1])
            extract_bd(g)
            b_gens[g] = phase_b(g)
        for _ in range(NB):
            run(b_gens[NG - 1])

    nc.finalize()
    return nc


_NC_CACHE = None


def _get_nc():
    global _NC_CACHE
    if _NC_CACHE is None:
        _NC_CACHE = build_bass()
    return _NC_CACHE


def kernel(Q: np.ndarray, K: np.ndarray, V: np.ndarray, mask: np.ndarray,
           _trace: bool = False):
    import ml_dtypes
    BF = ml_dtypes.bfloat16
    B, H = 4, 16
    NP = B * H
    per = NP // N_CORES
    ng_total = NP // 2
    # Q: rows permuted to (blk, j, p) order so the on-chip DMA-transpose
    # lands lhsT tiles directly; cols pair-interleaved (u, d).
    Qi = np.ascontiguousarray(
        np.asarray(Q, dtype=np.float32)
        .reshape(ng_total, 2, NB, 128, NJ, D)      # [g, u, blk, p, j, d]
        .transpose(0, 2, 4, 3, 1, 5)               # [g, blk, j, p, u, d]
        .reshape(ng_total, S, 128).astype(BF))
    # K: original s order, pair-interleaved [g, s, u, d]
    Ki = np.ascontiguousarray(
        np.asarray(K, dtype=np.float32).reshape(ng_total, 2, S, D)
        .transpose(0, 2, 1, 3).astype(BF))
    Vr = np.asarray(V, dtype=np.float32).reshape(NP, S, D)
    Mr = np.asarray(mask, dtype=np.float32).reshape(NP, S)
    # V packed with the mask column: exact for any mask, free when ones
    Vpk = np.empty((NP, S, E), dtype=BF)
    if np.all(Mr == 1.0):
        Vpk[:, :, 0:D] = Vr
    else:
        Vpk[:, :, 0:D] = Vr * Mr[:, :, None]
    Vpk[:, :, D] = Mr
    Vi = np.ascontiguousarray(
        Vpk.reshape(ng_total, 2, S, E).transpose(0, 2, 1, 3))

    in_maps = []
    gper = per // 2
    for i in range(N_CORES):
        sl = slice(i * gper, (i + 1) * gper)
        in_maps.append({
            "Q": np.ascontiguousarray(Qi[sl]),
            "K": np.ascontiguousarray(Ki[sl]),
            "V": np.ascontiguousarray(Vi[sl]),
        })

    nc = _get_nc()
    res = run_bass_kernel_spmd(nc, in_maps, core_ids=list(range(N_CORES)),
                               trace=_trace)
    # O per core: [NGc, S, 2, D] pair-interleaved, original s order
    out = np.concatenate(
        [np.asarray(r["O"]).astype(np.float32).transpose(0, 2, 1, 3)
         .reshape(per, S, D) for r in res.results], axis=0)
    if _trace:
        kernel._last_results = res
    return out.reshape(B, H, S, D)
